# revision 1
# baseline (speedup 1.0000x reference)
"""Farthest-point-sampling kernel for Trainium2 (Bass, single NeuronCore).

Sequential FPS over N=131072 3-D points, n_sample=4096, bit-exact vs the jax
reference (including first-occurrence argmax tie-breaking). All state lives in
SBUF; a hardware loop on the engine sequencers runs the 4095 selection steps:
ACT computes exact squared distances to the last winner, DVE does the running
min + two-level argmax (per-partition max/index, then cross-partition via PE
transposes), PE broadcasts the winner's coordinates via a ones-matmul, SP
streams chosen indices to DRAM in 63-element blocks.
"""

P, C, NSAMP = 128, 1024, 4096

import numpy as np
from concourse import bass, mybir
from concourse.alu_op_type import AluOpType

F32 = mybir.dt.float32
U32 = mybir.dt.uint32
I32 = mybir.dt.int32


def make_consts(P=128, C=1024):
    return {
        "identity": np.eye(P, dtype=np.float32),
        "iota_row": np.arange(P, dtype=np.float32).reshape(1, P),
        "ones_row": np.ones((1, P), dtype=np.float32),
    }


def build(P=128, C=1024, NSAMP=4096):
    N = P * C
    nc = bass.Bass("TRN2", target_bir_lowering=False)

    posD = nc.dram_tensor("pos", [N, 3], F32, kind="ExternalInput")
    idD = nc.dram_tensor("identity", [P, P], F32, kind="ExternalInput")
    iotaD = nc.dram_tensor("iota_row", [1, P], F32, kind="ExternalInput")
    onesD = nc.dram_tensor("ones_row", [1, P], F32, kind="ExternalInput")
    outD = nc.dram_tensor("out_idx", [NSAMP], I32, kind="ExternalOutput")

    import contextlib
    with contextlib.ExitStack() as _ctx:
        E = _ctx.enter_context
        block = E(nc.Block())
        dma_sem = E(nc.semaphore("dma_sem"))
        sem_act = E(nc.semaphore("sem_act"))      # +2/iter
        sem_s = E(nc.semaphore("sem_s"))          # +1 prologue, +1/iter
        sem_wc = E(nc.semaphore("sem_wc"))        # +1/iter
        sem_w = E(nc.semaphore("sem_w"))          # +1/iter
        sem_g = E(nc.semaphore("sem_g"))          # +1/iter
        sem_p1 = E(nc.semaphore("sem_p1"))        # +2/iter
        sem_p2 = E(nc.semaphore("sem_p2"))        # +1/iter
        sem_pbm = E(nc.semaphore("sem_pbm"))      # +1 prologue, +1/iter
        sem_pre = E(nc.semaphore("sem_pre"))      # +3 prologue
        posi = E(nc.sbuf_tensor("posi", [P, 3 * C], F32))
        xs = E(nc.sbuf_tensor("xs", [P, C], F32))
        ys = E(nc.sbuf_tensor("ys", [P, C], F32))
        zs = E(nc.sbuf_tensor("zs", [P, C], F32))
        d = E(nc.sbuf_tensor("d", [P, C], F32))
        sqx = E(nc.sbuf_tensor("sqx", [P, C], F32))
        sqy = E(nc.sbuf_tensor("sqy", [P, C], F32))
        sqz = E(nc.sbuf_tensor("sqz", [P, C], F32))
        t12 = E(nc.sbuf_tensor("t12", [P, C], F32))
        dist = E(nc.sbuf_tensor("dist", [P, C], F32))
        S = E(nc.sbuf_tensor("S", [P, 2], F32))
        m8 = E(nc.sbuf_tensor("m8", [P, 8], F32))
        ci8 = E(nc.sbuf_tensor("ci8", [P, 8], U32))
        ident = E(nc.sbuf_tensor("ident_sb", [P, P], F32))
        iota_row = E(nc.sbuf_tensor("iota_sb", [1, P], F32))
        ones_row = E(nc.sbuf_tensor("ones_sb", [1, P], F32))
        mv8 = E(nc.sbuf_tensor("mv8", [1, 8], F32))
        mi8 = E(nc.sbuf_tensor("mi8", [1, 8], U32))
        oh = E(nc.sbuf_tensor("oh", [1, P], F32))
        junk = E(nc.sbuf_tensor("junk", [1, P], F32))
        cstar_f = E(nc.sbuf_tensor("cstar_f", [1, 1], F32))
        cstar_u = E(nc.sbuf_tensor("cstar_u", [1, 1], U32))
        pf = E(nc.sbuf_tensor("pf", [1, 1], F32))
        gidx_f = E(nc.sbuf_tensor("gidx_f", [1, 1], F32))
        NITER_ = NSAMP - 1
        BOUT = 63 if NITER_ % 63 == 0 else NITER_
        NBLK = NITER_ // BOUT
        outbuf = E(nc.sbuf_tensor("outbuf", [1, BOUT], I32))
        wrow = E(nc.sbuf_tensor("wrow", [1, 3], F32))
        wcols = E(nc.sbuf_tensor("wcols", [P, 3], F32))
        bias128 = E(nc.sbuf_tensor("bias128", [P, 3], F32))
        zero_i = E(nc.sbuf_tensor("zero_i", [1, 1], I32))
        St0 = E(nc.psum_tensor("St0", [1, P], F32))
        St1 = E(nc.psum_tensor("St1", [1, P], F32))
        wc0 = E(nc.psum_tensor("wc0", [1, P], F32))
        wc1 = E(nc.psum_tensor("wc1", [1, P], F32))
        wc2 = E(nc.psum_tensor("wc2", [1, P], F32))
        biasP = E(nc.psum_tensor("biasP", [P, 3], F32))
        NITER = NSAMP - 1

        # ---------------- prologue ----------------
        @block.sync
        def _(sync):
            sync.dma_start(
                bass.AP(posi, 0, [[3 * C, P], [1, 3 * C]]),
                bass.AP(posD, 0, [[3 * C, P], [1, 3 * C]]),
            ).then_inc(dma_sem, 16)
            sync.dma_start(ident[:, :], idD[:, :]).then_inc(dma_sem, 16)
            sync.dma_start(iota_row[:, :], iotaD[:, :]).then_inc(dma_sem, 16)
            sync.dma_start(ones_row[:, :], onesD[:, :]).then_inc(dma_sem, 16)

        @block.vector
        def _(vector):
            vector.wait_ge(dma_sem, 64)
            for k, t in ((0, xs), (1, ys), (2, zs)):
                vector.tensor_copy(
                    t[:, :], bass.AP(posi, k, [[3 * C, P], [3, C]])
                ).then_inc(sem_pre, 1)
            vector.memset(d[:, :], 3.4028234663852886e38)
            vector.memset(zero_i[:, :], 0)
            vector.drain()
            vector.engine_nop().then_inc(sem_s, 1)  # zero_i ready

        @block.tensor
        def _(tensor):
            tensor.wait_ge(dma_sem, 64)
            # initial bias: coords of point 0 broadcast to all partitions
            tensor.matmul(
                biasP[:, 0:3], ones_row[:, :], posi[0:1, 0:3]
            ).then_inc(sem_pbm, 1)

        @block.sync
        def _(sync):
            sync.wait_ge(sem_s, 1)
            sync.dma_start(outD[0:1], zero_i[0:1, 0:1]).then_inc(dma_sem, 16)

        # ---------------- main loop ----------------
        @block.scalar
        def _(scalar):
            scalar.wait_ge(sem_pre, 3)
            with scalar.Fori(0, NITER) as u:
                scalar.wait_ge(sem_pbm, u + 1)
                scalar.copy(bias128[:, :], biasP[:, :])
                scalar.drain()
                scalar.activation(
                    sqx[:, :], xs[:, :], mybir.ActivationFunctionType.Square,
                    bias=bias128[:, 0:1], scale=-1.0,
                )
                scalar.activation(
                    sqy[:, :], ys[:, :], mybir.ActivationFunctionType.Square,
                    bias=bias128[:, 1:2], scale=-1.0,
                ).then_inc(sem_act, 1)
                scalar.activation(
                    sqz[:, :], zs[:, :], mybir.ActivationFunctionType.Square,
                    bias=bias128[:, 2:3], scale=-1.0,
                ).then_inc(sem_act, 1)

        @block.vector
        def _(vector):
            regs_ctx = (vector.register("regc"), vector.register("slot"),
                        vector.register("thr"))
            import contextlib as _cl
            with _cl.ExitStack() as _rs:
                regc, slot, thr = [_rs.enter_context(r) for r in regs_ctx]
                vector.reg_mov(slot, 0)
                vector.reg_mov(thr, 80)
                _loop = _rs.enter_context(vector.Fori(0, NITER))
                u = _loop
                vector.wait_ge(sem_act, 2 * u + 1)
                vector.drain()
                vector.tensor_tensor(
                    t12[:, :], sqx[:, :], sqy[:, :], AluOpType.add
                )
                vector.drain()
                vector.wait_ge(sem_act, 2 * u + 2)
                vector.tensor_tensor(
                    dist[:, :], t12[:, :], sqz[:, :], AluOpType.add
                )
                vector.drain()
                vector.tensor_tensor(
                    d[:, :], d[:, :], dist[:, :], AluOpType.min
                )
                vector.drain()
                vector.tensor_reduce(
                    S[:, 0:1], d[:, :], axis=mybir.AxisListType.X,
                    op=AluOpType.max,
                )
                vector.drain()
                vector.tensor_copy(m8[:, :], bass.AP(S, 0, [[2, P], [0, 8]]))
                vector.drain()
                vector.max_index(ci8[:, :], m8[:, :], d[:, :])
                vector.drain()
                vector.tensor_copy(S[:, 1:2], ci8[:, 0:1]).then_inc(sem_s, 1)
                vector.drain()
                # cross-partition argmax on transposed m row
                vector.wait_ge(sem_p1, 2 * u + 1)
                vector.max(mv8[:, :], St0[0:1, :])
                vector.drain()
                vector.max_index(mi8[:, :], mv8[:, :], St0[0:1, :])
                vector.drain()
                vector.tensor_copy(pf[:, :], mi8[0:1, 0:1])
                vector.drain()
                vector.tensor_scalar(
                    oh[:, :], iota_row[:, :], pf[0:1, 0:1], None,
                    op0=AluOpType.is_equal,
                )
                vector.drain()
                # c* = sum(St1 * oh)
                vector.wait_ge(sem_p1, 2 * u + 2)
                vector.tensor_tensor(
                    junk[:, :], St1[0:1, :], oh[:, :], AluOpType.mult
                )
                vector.drain()
                vector.tensor_reduce(
                    cstar_f[:, :], junk[:, :], axis=mybir.AxisListType.X,
                    op=AluOpType.add,
                )
                vector.drain()
                vector.tensor_copy(cstar_u[:, :], cstar_f[:, :])
                vector.drain()
                vector.load(regc, cstar_u[0:1, 0:1])
                vector.reg_alu(regc, regc, 3, op=AluOpType.mult)
                c3 = vector.snap(regc, min_val=0, max_val=3 * (C - 1))
                vector.tensor_copy(
                    wcols[:, :], posi[:, bass.ds(c3, 3)]
                ).then_inc(sem_wc, 1)
                vector.drain()
                vector.tensor_scalar(
                    gidx_f[:, :], pf[:, :], float(C), cstar_f[0:1, 0:1],
                    op0=AluOpType.mult, op1=AluOpType.add,
                )
                vector.drain()
                vector.wait_ge(dma_sem, thr)
                slot_v = vector.snap(slot, min_val=0, max_val=BOUT - 1)
                vector.tensor_copy(
                    outbuf[0:1, bass.ds(slot_v, 1)], gidx_f[:, :]
                ).then_inc(sem_g, 1)
                vector.drain()
                vector.reg_alu(slot, slot, 1, op=AluOpType.add)
                with vector.If_cmp(slot, BOUT, "IS_EQ"):
                    vector.reg_mov(slot, 0)
                    vector.reg_alu(thr, thr, 16, op=AluOpType.add)
                # winner coords: wrow[k] = sum(wck * oh)
                vector.wait_ge(sem_p2, u + 1)
                for k in range(3):
                    wck = (wc0, wc1, wc2)[k]
                    vector.tensor_tensor(
                        junk[:, :], wck[0:1, :], oh[:, :], AluOpType.mult
                    )
                    vector.drain()
                    ins = vector.tensor_reduce(
                        wrow[0:1, k : k + 1], junk[:, :],
                        axis=mybir.AxisListType.X, op=AluOpType.add,
                    )
                    if k == 2:
                        ins.then_inc(sem_w, 1)
                    vector.drain()

        @block.tensor
        def _(tensor):
            with tensor.Fori(0, NITER) as u:
                tensor.wait_ge(sem_s, u + 2)
                tensor.transpose(St0[:, :], S[:, 0:1], ident[:, :]).then_inc(
                    sem_p1, 1
                )
                tensor.transpose(St1[:, :], S[:, 1:2], ident[:, :]).then_inc(
                    sem_p1, 1
                )
                tensor.wait_ge(sem_wc, u + 1)
                tensor.transpose(wc0[:, :], wcols[:, 0:1], ident[:, :])
                tensor.transpose(wc1[:, :], wcols[:, 1:2], ident[:, :])
                tensor.transpose(wc2[:, :], wcols[:, 2:3], ident[:, :]).then_inc(
                    sem_p2, 1
                )
                tensor.wait_ge(sem_w, u + 1)
                tensor.matmul(
                    biasP[:, 0:3], ones_row[:, :], wrow[0:1, 0:3]
                ).then_inc(sem_pbm, 1)

        @block.sync
        def _(sync):
            with sync.Fori(0, NBLK) as v:
                sync.wait_ge(sem_g, BOUT * v + BOUT)
                sync.dma_start(
                    outD[bass.ds(BOUT * v + 1, BOUT)], outbuf[0:1, 0:BOUT]
                ).then_inc(dma_sem, 16)

        @block.sync
        def _(sync):
            sync.wait_ge(dma_sem, 80 + 16 * NBLK)

    return nc




_CACHE = {}


def kernel(pos):
    import numpy as np
    from concourse.bass_utils import run_bass_kernel_spmd

    pos = np.ascontiguousarray(np.asarray(pos, dtype=np.float32))
    assert pos.shape == (P * C, 3)
    if "nc" not in _CACHE:
        _CACHE["nc"] = build(P=P, C=C, NSAMP=NSAMP)
    nc = _CACHE["nc"]
    in_map = {"pos": pos}
    in_map.update(make_consts(P, C))
    res = run_bass_kernel_spmd(nc, [in_map], core_ids=[0])
    out = res.results[0]["out_idx"].astype(np.int32)
    return out

